# revision 4
# baseline (speedup 1.0000x reference)
"""RIENet loss kernel (keypoint/KNN MSE + global-align Huber-min loss) on 8 trn2 cores.

Sharding: core ci -> (b = ci // 4, n-chunk j = ci % 4).  Each core holds the full
tgt[b] (M=8192 points) and a 2048-column chunk of src_transformed[b] (N axis).
  loss_1 (min over M per src point): complete locally per core.
  loss_2 (min over N per tgt point): per-core partial min over its chunk;
          host min-reduces the 4 chunks per batch element.

Device kernel per core:
  P[m, n] = ||t_m - s_n||^2 computed directly by one K=9 fp32 matmul with
  augmented operands  lhsT rows = [t, t^2, 1],  rhs rows = [-2 s, 1, s^2].
  For each 128-row m-tile (PSUM 128x2048 across 4 banks):
    - DVE tensor_tensor(min) into a column accumulator  (min over m-tiles)
    - ACT copies the odd half PSUM->SBUF; DVE tensor_tensor_reduce(min,min)
      over (even_half, odd_half) emits the exact per-row chunk min in one pass.
  Column accumulator is partition-min-reduced via PE transposes + DVE reduces.
  The tiny keypoint / KNN MSE losses run on-device as well (redundantly on
  every core; host reads core 0's copy).
"""

import os
import numpy as np


def _ensure_path():
    try:
        import concourse  # noqa: F401
    except ImportError:
        import sys
        for p in ("/opt/trn_rl_repo", "/root/.axon_site/_ro/trn_rl_repo"):
            if os.path.isdir(p) and p not in sys.path:
                sys.path.insert(0, p)


_ensure_path()

import concourse.bass as bass  # noqa: E402
import concourse.bacc as bacc  # noqa: E402
import concourse.tile as tile  # noqa: E402
import concourse.mybir as mybir  # noqa: E402
from concourse.bass_utils import run_bass_kernel_spmd  # noqa: E402

F32 = mybir.dt.float32
AL = mybir.AluOpType
AF = mybir.ActivationFunctionType

MARGIN = 0.1
B, KP, KNN, N, M = 2, 256, 32, 8192, 8192
NCORES = 8
NSHARDS = NCORES // B          # 4 n-chunks per batch element
CHUNK = N // NSHARDS           # 2048
HALF = CHUNK // 2              # 1024
NJ = CHUNK // 512              # 4 psum banks per m-tile
MI = M // 128                  # 64 m-tiles
BIG = 3.0e38

_CACHE = {}


def _build():
    nc = bacc.Bacc("TRN2", target_bir_lowering=False, debug=False,
                   num_devices=NCORES)

    src = nc.dram_tensor("src", [3, CHUNK], F32, kind="ExternalInput")
    tgt = nc.dram_tensor("tgt", [3, M], F32, kind="ExternalInput")
    ident = nc.dram_tensor("ident", [128, 128], F32, kind="ExternalInput")
    kp_lhsT = nc.dram_tensor("kp_lhsT", [4, 2 * 3], F32, kind="ExternalInput")
    kp_rhs = nc.dram_tensor("kp_rhs", [4, 2 * KP], F32, kind="ExternalInput")
    tgt_kp = nc.dram_tensor("tgt_kp", [3, 2 * KP], F32, kind="ExternalInput")
    knn_src = nc.dram_tensor("knn_src", [128, 2 * 192], F32, kind="ExternalInput")
    knn_tgt = nc.dram_tensor("knn_tgt", [128, 2 * 192], F32, kind="ExternalInput")

    colmin_o = nc.dram_tensor("colmin", [128, CHUNK // 128], F32, kind="ExternalOutput")
    rowmin_o = nc.dram_tensor("rowmin", [128, MI], F32, kind="ExternalOutput")
    misc_o = nc.dram_tensor("misc", [128, 4], F32, kind="ExternalOutput")

    with tile.TileContext(nc) as tc:
        with (
            tc.tile_pool(name="const", bufs=1) as const,
            tc.tile_pool(name="sc", bufs=3) as sc,
        ):
            aug_t = const.tile([9, M], F32)        # [t; t^2; 1]
            aug_s = const.tile([9, CHUNK], F32)    # [-2s; 1; s^2]
            acc = const.tile([128, CHUNK], F32)    # running min over m-tiles
            rowbuf = const.tile([128, MI], F32)    # per-row chunk mins
            id_sb = const.tile([128, 128], F32)
            colmin_sb = const.tile([128, CHUNK // 128], F32)
            misc_sb = const.tile([128, 4], F32)
            junk = const.tile([128, 1], F32)       # broadcast sink for ttr out

            # Engine ops must address SBUF at partition offset 0, so build the
            # partition-3..8 rows of the augmented operands in offset-0 staging
            # tiles and DMA them into place (DMA can write any partition).
            stage_t = const.tile([3, M], F32)
            stage_s = const.tile([3, CHUNK], F32)

            nc.sync.dma_start(out=aug_t[0:3, :], in_=tgt[:])
            nc.sync.dma_start(out=aug_s[0:3, :], in_=src[:])
            nc.sync.dma_start(out=id_sb[:], in_=ident[:])
            nc.scalar.mul(out=aug_s[0:3, :], in_=aug_s[0:3, :], mul=-2.0)

            nc.sync.dma_start(out=stage_t[:], in_=tgt[:])
            nc.scalar.activation(out=stage_t[:], in_=stage_t[:], func=AF.Square)
            nc.sync.dma_start(out=aug_t[3:6, :], in_=stage_t[:])
            nc.vector.memset(stage_t[:], 1.0)
            nc.sync.dma_start(out=aug_t[6:9, :], in_=stage_t[:])

            nc.sync.dma_start(out=stage_s[:], in_=src[:])
            nc.scalar.activation(out=stage_s[:], in_=stage_s[:], func=AF.Square)
            nc.sync.dma_start(out=aug_s[6:9, :], in_=stage_s[:])
            nc.vector.memset(stage_s[:], 1.0)
            nc.sync.dma_start(out=aug_s[3:6, :], in_=stage_s[:])

            nc.gpsimd.memset(acc[:], BIG)
            nc.gpsimd.memset(misc_sb[:], 0.0)

            with tc.tile_pool(name="psum_main", bufs=2, space="PSUM") as pm:
                for mi in range(MI):
                    pt = pm.tile([128, CHUNK], F32, tag="pt")
                    for nj in range(NJ):
                        nc.tensor.matmul(
                            pt[:, nj * 512:(nj + 1) * 512],
                            lhsT=aug_t[:, mi * 128:(mi + 1) * 128],
                            rhs=aug_s[:, nj * 512:(nj + 1) * 512],
                            start=True, stop=True,
                        )
                    # min over m-tiles (elementwise, per n column)
                    nc.vector.tensor_tensor(out=acc[:], in0=pt[:], in1=acc[:],
                                            op=AL.min)
                    # per-row min over the whole 2048-chunk in one DVE pass:
                    # pair even/odd halves via a min/min scan whose broadcast
                    # output leaves the final (complete) min in rowbuf[:, mi].
                    odd = sc.tile([128, HALF], F32, tag="odd")
                    nc.scalar.copy(out=odd[:], in_=pt[:, HALF:])
                    nc.vector.tensor_tensor_scan(
                        out=rowbuf[:, mi:mi + 1].broadcast_to((128, HALF)),
                        data0=pt[:, 0:HALF],
                        data1=odd[:],
                        initial=BIG,
                        op0=AL.min,
                        op1=AL.min,
                    )

            with tc.tile_pool(name="psum_fin", bufs=2, space="PSUM") as pf:
                # partition-axis min of acc via PE transposes
                for blk in range(CHUNK // 128):
                    tp = pf.tile([128, 128], F32, tag="tp")
                    nc.tensor.transpose(tp[:], acc[:, blk * 128:(blk + 1) * 128],
                                        id_sb[:])
                    nc.vector.tensor_reduce(
                        out=colmin_sb[:, blk:blk + 1], in_=tp[:],
                        axis=mybir.AxisListType.X, op=AL.min)

                # tiny keypoint / knn losses (both batch elements)
                kp_l = const.tile([4, 2 * 3], F32)
                kp_r = const.tile([4, 2 * KP], F32)
                kp_t = const.tile([3, 2 * KP], F32)
                ks = const.tile([128, 2 * 192], F32)
                kt = const.tile([128, 2 * 192], F32)
                nc.sync.dma_start(out=kp_l[:], in_=kp_lhsT[:])
                nc.sync.dma_start(out=kp_r[:], in_=kp_rhs[:])
                nc.sync.dma_start(out=kp_t[:], in_=tgt_kp[:])
                nc.sync.dma_start(out=ks[:], in_=knn_src[:])
                nc.sync.dma_start(out=kt[:], in_=knn_tgt[:])
                for b in range(B):
                    # R @ src_kp + t  via K=4 matmul (lhsT = [R^T; t])
                    pt2 = pf.tile([3, KP], F32, tag="kp")
                    nc.tensor.matmul(
                        pt2[:], lhsT=kp_l[:, b * 3:(b + 1) * 3],
                        rhs=kp_r[:, b * KP:(b + 1) * KP],
                        start=True, stop=True)
                    diff = sc.tile([3, KP], F32, tag="kdiff")
                    nc.vector.tensor_sub(diff[:], pt2[:],
                                         kp_t[:, b * KP:(b + 1) * KP])
                    nc.vector.tensor_mul(diff[:], diff[:], diff[:])
                    nc.vector.tensor_reduce(
                        out=misc_sb[0:3, b:b + 1], in_=diff[:],
                        axis=mybir.AxisListType.X, op=AL.add)
                    diff2 = sc.tile([128, 192], F32, tag="ndiff")
                    nc.vector.tensor_sub(diff2[:], ks[:, b * 192:(b + 1) * 192],
                                         kt[:, b * 192:(b + 1) * 192])
                    nc.vector.tensor_mul(diff2[:], diff2[:], diff2[:])
                    nc.vector.tensor_reduce(
                        out=misc_sb[:, 2 + b:3 + b], in_=diff2[:],
                        axis=mybir.AxisListType.X, op=AL.add)

            nc.sync.dma_start(out=colmin_o[:], in_=colmin_sb[:])
            nc.sync.dma_start(out=rowmin_o[:], in_=rowbuf[:])
            nc.sync.dma_start(out=misc_o[:], in_=misc_sb[:])

    nc.compile()
    return nc


def _get_nc():
    if "nc" not in _CACHE:
        _CACHE["nc"] = _build()
    return _CACHE["nc"]


def _prepare_in_maps(src_keypoints, tgt_keypoints, rotation_ab, translation_ab,
                     src_keypoints_knn, tgt_keypoints_knn, src_transformed, tgt):
    f = np.float32
    st = np.ascontiguousarray(np.asarray(src_transformed, dtype=f))
    tg = np.ascontiguousarray(np.asarray(tgt, dtype=f))
    skp = np.asarray(src_keypoints, dtype=f)
    tkp = np.asarray(tgt_keypoints, dtype=f)
    rot = np.asarray(rotation_ab, dtype=f)
    tra = np.asarray(translation_ab, dtype=f)
    sknn = np.asarray(src_keypoints_knn, dtype=f)
    tknn = np.asarray(tgt_keypoints_knn, dtype=f)

    ident = np.eye(128, dtype=f)
    kp_lhsT = np.zeros((4, 2 * 3), dtype=f)
    kp_rhs = np.zeros((4, 2 * KP), dtype=f)
    tgt_kp = np.zeros((3, 2 * KP), dtype=f)
    knn_src = np.zeros((128, 2 * 192), dtype=f)
    knn_tgt = np.zeros((128, 2 * 192), dtype=f)
    for b in range(B):
        kp_lhsT[0:3, b * 3:(b + 1) * 3] = rot[b].T
        kp_lhsT[3, b * 3:(b + 1) * 3] = tra[b]
        kp_rhs[0:3, b * KP:(b + 1) * KP] = skp[b]
        kp_rhs[3, b * KP:(b + 1) * KP] = 1.0
        tgt_kp[:, b * KP:(b + 1) * KP] = tkp[b]
        knn_src[:, b * 192:(b + 1) * 192] = sknn[b].reshape(128, 192)
        knn_tgt[:, b * 192:(b + 1) * 192] = tknn[b].reshape(128, 192)

    shared = {
        "ident": ident, "kp_lhsT": kp_lhsT, "kp_rhs": kp_rhs,
        "tgt_kp": tgt_kp, "knn_src": knn_src, "knn_tgt": knn_tgt,
    }
    in_maps = []
    for ci in range(NCORES):
        b, j = divmod(ci, NSHARDS)
        m = dict(shared)
        m["src"] = np.ascontiguousarray(st[b, :, j * CHUNK:(j + 1) * CHUNK])
        m["tgt"] = tg[b]
        in_maps.append(m)
    return in_maps


def _huber(x, c):
    return np.where(x < c, 0.5 * x * x, c * x - 0.5 * c * c)


def _postprocess(results):
    c = np.float64(MARGIN)
    loss1 = np.float64(0.0)
    loss2 = np.float64(0.0)
    for b in range(B):
        rowmins = []
        for j in range(NSHARDS):
            r = results[b * NSHARDS + j]
            colmin = np.asarray(r["colmin"], dtype=np.float64).T.ravel()
            loss1 += _huber(colmin, c).sum()
            rowmins.append(np.asarray(r["rowmin"], dtype=np.float64).T.ravel())
        rm = np.minimum.reduce(rowmins)
        loss2 += _huber(rm, c).sum()
    gal = loss1 + loss2

    misc = np.asarray(results[0]["misc"], dtype=np.float64)
    kp_loss = (misc[0:3, 0].sum() + misc[0:3, 1].sum()) / B
    knn_loss = (misc[:, 2].sum() + misc[:, 3].sum()) / (B * KNN)
    ncl = knn_loss + kp_loss
    return np.float32(ncl), np.float32(gal)


def run_device(in_maps, **kw):
    nc = _get_nc()
    return run_bass_kernel_spmd(nc, in_maps, list(range(NCORES)), **kw)


def kernel(src_keypoints, tgt_keypoints, rotation_ab, translation_ab,
           src_keypoints_knn, tgt_keypoints_knn, k, src_transformed, tgt,
           **_unused):
    in_maps = _prepare_in_maps(src_keypoints, tgt_keypoints, rotation_ab,
                               translation_ab, src_keypoints_knn,
                               tgt_keypoints_knn, src_transformed, tgt)
    res = run_device(in_maps)
    return _postprocess(res.results)
